# revision 51
# baseline (speedup 1.0000x reference)
"""AdaGAE forward on 8 Trainium2 NeuronCores.

reference:
    h   = relu(spmm(X @ W1))        # spmm = COO Laplacian x dense
    emb = spmm(h @ W2)
    out = softmax(-pairwise_sqdist(emb), axis=1) + 1e-10   # [8192, 8192]

Sharding: nodes row-sharded, 1024 rows/core. Each spmm gathers rows of the
all-gathered table with dma_gather and reduces per-chunk with a one-hot
matmul G.T @ S that also transposes (h and emb are kept feature-major).
Numerics: gathered tables (Y1, Y2) and the one-hot S matrices are fp8-e4m3
(host-validated L2 ~2e-4 vs the 2e-2 gate; the softmax is near-one-hot so
the GNN front-end has huge precision headroom); the NxN phase stays bf16
with f32 PSUM accumulation, sq computed from the SAME quantized embedding
so the diagonal exp(0)=1 cancellation is exact. exp(z - sq_i) runs on
ScalarE writing a bf16 stripe (row max of -dist is exactly 0, bias=-sq_i
f32), row sums split ScalarE-accum/VectorE-reduce, one VectorE normalize
(*1/s + 1e-10), DMA out bf16 (host converts to f32).

Perf structure (v0 was 384us):
 - spmm dst-blocks are 256 wide: each LDWEIGHTS (128c) feeds a 256-col
   matmul; 1:1 LDW:MM streams never release the PE HAM clock gate
   (it only counts MATMUL activity), so the whole stream ran at 1.2GHz
 - fp8 tables halve the AG1 wire and the gather bytes (swdge queues are
   ~60GB/s each); one dma_gather per 256-block = 4 gathers on 4 queues
 - filler matmuls keyed on each collective's output re-warm the clock
   during the gather lead-in of the next phase
 - output stored bf16 (halves the 33.5MB/core output DMA)
 - phase A runs k-outer so matmuls start on the first k-chunk DMA
"""

import dataclasses
import os
import sys

if "/opt/trn_rl_repo" not in sys.path:
    sys.path.insert(0, "/opt/trn_rl_repo")

import ml_dtypes
import numpy as np

import concourse.bacc as bacc
import concourse.bass as bass
import concourse.mybir as mybir
from concourse.tile import TileContext
from concourse.bass_utils import run_bass_kernel_spmd

NC = 8          # cores
N = 8192        # nodes
R = N // NC     # rows per core
P = 128
BW = 256        # spmm dst-block width
DIN = 1024
DMID = 256
DEMB = 64
KC = DIN // P   # k-chunks for X @ W1
NBLK = R // P   # 128-row blocks per core (py2 / tails / E stripes)
NB2 = R // BW   # 256-row spmm blocks per core
Y2W = 256       # Y2 table row: 256 fp8 cols (gather rows must be 256B)

F32 = mybir.dt.float32
BF16 = mybir.dt.bfloat16
F8 = mybir.dt.float8e4
I16 = mybir.dt.int16
BFNP = ml_dtypes.bfloat16
F8NP = mybir.dt.np(F8)

LAST_RESULTS = None  # BassKernelResults of the most recent run (for test.py)

_GRAPH_CACHE = {}

# filler matmuls after each collective to re-warm the PE clock before the
# next phase's matmul stream (sized to the gather lead-in window)
W_AG1 = int(os.environ.get("W_AG1", "100"))
W_AG2 = int(os.environ.get("W_AG2", "80"))
W_AG3 = int(os.environ.get("W_AG3", "60"))


def _build(b_ch: int):
    """Build the per-core Bass graph. b_ch = gather chunks per 256-row block."""
    kphases = int(os.environ.get("KPHASES", "9"))
    c1 = NB2 * b_ch           # total chunks per core
    ne = c1 * P               # padded gather rows per core
    nc = bacc.Bacc(None, target_bir_lowering=False, debug=False, num_devices=NC,
                   num_swdge_queues=4)

    xT = nc.declare_dram_parameter("xT", [DIN, R], BF16, isOutput=False)
    w1 = nc.declare_dram_parameter("w1", [DIN, DMID], BF16, isOutput=False)
    w2 = nc.declare_dram_parameter("w2", [DMID, DEMB], BF16, isOutput=False)
    idx = nc.declare_dram_parameter("idx", [128, ne // 16], I16, isOutput=False)
    smat = nc.declare_dram_parameter("smat", [128, c1 * BW], F8, isOutput=False)
    out = nc.declare_dram_parameter("out", [R, N], BF16, isOutput=True)

    ag1_in = nc.dram_tensor("ag1_in", [R, DMID], F8)
    ag1_out = nc.dram_tensor("ag1_out", [N, DMID], F8, addr_space="Shared")
    ag2_in = nc.dram_tensor("ag2_in", [R, Y2W], F8)
    ag2_out = nc.dram_tensor("ag2_out", [N, Y2W], F8, addr_space="Shared")
    ag3_in = nc.dram_tensor("ag3_in", [DEMB + 2, R], BF16)
    ag3_out = nc.dram_tensor("ag3_out", [NC * (DEMB + 2), R], BF16,
                             addr_space="Shared")

    rg = [list(range(NC))]

    with (
        TileContext(nc) as tc,
        tc.tile_pool(name="sb", bufs=1) as sb,
    ):
        # ---- phase A: Y1 = X @ W1 (row-sharded) -> ag1_in (fp8) ------------
        # k-outer over 2 halves of 4 blocks so matmuls start on chunk 0
        with (
            tc.tile_pool(name="pa", bufs=1) as pa,
            tc.tile_pool(name="pap", bufs=1, space="PSUM") as pap,
        ):
            xts, w1s = [], []
            for k in range(KC):
                xt_t = pa.tile([P, R], BF16, tag="xt", bufs=KC, name=f"xt_{k}")
                nc.sync.dma_start(out=xt_t[:], in_=xT[k * P:(k + 1) * P, :])
                xts.append(xt_t)
                w1_t = pa.tile([P, DMID], BF16, tag="w1", bufs=KC, name=f"w1_{k}")
                nc.sync.dma_start(out=w1_t[:], in_=w1[k * P:(k + 1) * P, :])
                w1s.append(w1_t)

            # persistent small loads next, the s_all table last (only phase B
            # needs it; keeps it off phase A's critical DMA path)
            idx_t = sb.tile([128, ne // 16], I16, tag="idx", bufs=1)
            nc.sync.dma_start(out=idx_t[:], in_=idx[:, :])
            w2a = sb.tile([P, DEMB], BF16, tag="w2a", bufs=1)
            nc.sync.dma_start(out=w2a[:], in_=w2[0:P, :])
            w2b = sb.tile([P, DEMB], BF16, tag="w2b", bufs=1)
            nc.sync.dma_start(out=w2b[:], in_=w2[P:2 * P, :])
            s_all = sb.tile([P, c1 * BW], F8, tag="sall", bufs=1)
            nc.sync.dma_start(out=s_all[:], in_=smat[:, :])

            for half in range(2):
                ms = range(half * 4, half * 4 + 4)
                py1s = {m: pap.tile([P, DMID], F32, tag="py1", bufs=8,
                                    name=f"py1_{m}") for m in ms}
                for k in range(KC):
                    for m in ms:
                        nc.tensor.matmul(
                            out=py1s[m][:],
                            lhsT=xts[k][:, m * P:(m + 1) * P],
                            rhs=w1s[k][:],
                            start=(k == 0),
                            stop=(k == KC - 1),
                        )
                for m in ms:
                    y1 = pa.tile([P, DMID], F8, tag="y1", bufs=2,
                                 name=f"y1_{m}")
                    nc.scalar.copy(out=y1[:], in_=py1s[m][:])
                    nc.sync.dma_start(out=ag1_in[m * P:(m + 1) * P, :],
                                      in_=y1[:])

        nc.gpsimd.collective_compute(
            "AllGather", mybir.AluOpType.bypass, replica_groups=rg,
            ins=[ag1_in.ap().opt()], outs=[ag1_out.ap().opt()],
        )

        ps = tc.alloc_tile_pool(name="ps", bufs=1, space="PSUM")

        def bank1(name):
            return ps.tile([P, 512], F32, tag="bank1", bufs=6, name=name)

        def warm(n, tag, lhsT_t, rhs_tile, kdim, cols):
            # filler matmuls: re-warm the PE HAM clock (K=8) right after a
            # collective completes, bridging the gather lead-in before the
            # next phase's real matmuls. rhs_tile is a small SBUF tile DMA'd
            # from the collective's output, so these cannot start earlier.
            for i in range(n):
                dps = bank1(f"warm_{tag}_{i}")
                nc.tensor.matmul(
                    out=dps[0:DEMB, 0:cols],
                    lhsT=lhsT_t[0:kdim, 0:DEMB],
                    rhs=rhs_tile[0:kdim, 0:cols],
                    start=True, stop=True,
                )

        wt1 = sb.tile([P, DMID], F8, tag="wt1", bufs=1)
        nc.sync.dma_start(out=wt1[:], in_=ag1_out[0:P, :])
        warm(W_AG1, "ag1", wt1, s_all, P, 512)

        # gathers: 4 sub-gathers per block round-robined over the 4 swdge
        # queues — descriptor generation serializes per instruction
        # (~8ns/row), so big single gathers block the later queues
        gsub = [(b_ch + 3) // 4] * 3
        gsub.append(b_ch - sum(gsub))

        def block_gather(pool, pfx, b, k0, table_ap, esize, estep, qoff):
            g = pool.tile([P, b_ch, esize], F8, tag=pfx, bufs=NB2,
                          name=f"{pfx}_{b}")
            j0 = 0
            for si, nj in enumerate(gsub):
                if nj <= 0:
                    continue
                nc.gpsimd.dma_gather(
                    out_ap=g[:, j0:j0 + nj, :],
                    in_ap=table_ap,
                    idxs_ap=idx_t[:, (k0 + j0) * 8:(k0 + j0 + nj) * 8],
                    num_idxs=nj * P,
                    num_idxs_reg=nj * P,
                    elem_size=esize,
                    elem_step=estep,
                    single_packet=False,
                    queue_num=(qoff + si) % 4,
                )
                j0 += nj
            return g

        if kphases >= 2:
            # ---- phase B: hT = relu(A @ Y1).T, feature-major [256, 1024] ---
            ht0 = sb.tile([P, R], BF16, tag="ht0", bufs=1)
            ht1 = sb.tile([P, R], BF16, tag="ht1", bufs=1)
            with tc.tile_pool(name="pb", bufs=1) as pb:
                g1s = []
                for b in range(NB2):
                    g1s.append(block_gather(pb, "g1", b, b * b_ch,
                                            ag1_out[:, :], DMID, DMID, 0))
                for b in range(NB2):
                    k0 = b * b_ch
                    g1 = g1s[b]
                    pha = bank1(f"pha_{b}")
                    phb = bank1(f"phb_{b}")
                    for j in range(b_ch):
                        k = k0 + j
                        nc.tensor.matmul(
                            out=pha[:, 0:BW],
                            lhsT=g1[:, j, 0:P],
                            rhs=s_all[:, k * BW:(k + 1) * BW],
                            start=(j == 0), stop=(j == b_ch - 1),
                        )
                    for j in range(b_ch):
                        k = k0 + j
                        nc.tensor.matmul(
                            out=phb[:, 0:BW],
                            lhsT=g1[:, j, P:2 * P],
                            rhs=s_all[:, k * BW:(k + 1) * BW],
                            start=(j == 0), stop=(j == b_ch - 1),
                        )
                    nc.scalar.activation(
                        out=ht0[:, b * BW:(b + 1) * BW], in_=pha[:, 0:BW],
                        func=mybir.ActivationFunctionType.Relu,
                    )
                    nc.scalar.activation(
                        out=ht1[:, b * BW:(b + 1) * BW], in_=phb[:, 0:BW],
                        func=mybir.ActivationFunctionType.Relu,
                    )
                # phase C: Y2 blocks -> ag2_in (deferred so the spmm matmul
                # stream above runs back-to-back on TensorE)
                for b in range(NBLK):
                    py2 = bank1(f"py2_{b}")
                    nc.tensor.matmul(
                        out=py2[:, 0:DEMB], lhsT=ht0[:, b * P:(b + 1) * P],
                        rhs=w2a[:], start=True, stop=False,
                    )
                    nc.tensor.matmul(
                        out=py2[:, 0:DEMB], lhsT=ht1[:, b * P:(b + 1) * P],
                        rhs=w2b[:], start=False, stop=True,
                    )
                    y2 = pb.tile([P, Y2W], F8, tag="y2", bufs=2, name=f"y2_{b}")
                    nc.scalar.copy(out=y2[:, 0:DEMB], in_=py2[:, 0:DEMB])
                    nc.vector.memset(y2[:, DEMB:Y2W], 0)
                    nc.sync.dma_start(out=ag2_in[b * P:(b + 1) * P, :], in_=y2[:])

        if kphases >= 3:
            nc.gpsimd.collective_compute(
                "AllGather", mybir.AluOpType.bypass, replica_groups=rg,
                ins=[ag2_in.ap().opt()], outs=[ag2_out.ap().opt()],
            )

        wt2 = sb.tile([P, Y2W], F8, tag="wt2", bufs=1)
        nc.sync.dma_start(out=wt2[:], in_=ag2_out[0:P, :])
        warm(W_AG2, "ag2", wt2, s_all, P, 512)

        if kphases >= 4:
            # ---- phase D: embT = (A @ Y2).T; c_loc = [embT; -sqh; -sql] ----
            c_loc = sb.tile([DEMB, R], BF16, tag="cloc", bufs=1)
            # lhsT padded to 128 contraction rows (66 real + 62 zeros): a
            # 66-row stationary leaves the PE array half-loaded and the HAM
            # clock gate never releases (K=4 = 1.2GHz) for the whole N x N
            # phase. Zero rows cost no extra matmul cycles.
            lhsT_all = sb.tile([P, R], BF16, tag="lhsT", bufs=1)
            # partition base must be a multiple of 32: zero rows 64:128 now,
            # the ones-rows 64:66 are written after phase D's tails
            nc.vector.memset(lhsT_all[DEMB:P, :], 0)
            bias_t = sb.tile([P, NBLK], F32, tag="bias", bufs=1)
            with tc.tile_pool(name="pd", bufs=1) as pd:
                negones = pd.tile([DEMB, 1], F32, tag="negones", bufs=1)
                nc.vector.memset(negones[:], -1.0)
                one1 = pd.tile([1, 1], F32, tag="one1", bufs=1)
                nc.vector.memset(one1[:], 1.0)
                sqrow = pd.tile([1, R], F32, tag="sqrow", bufs=1)
                sqh_b = pd.tile([1, R], BF16, tag="sqh_b", bufs=1)
                sql_b = pd.tile([1, R], BF16, tag="sql_b", bufs=1)
                g2s = []
                for b in range(NB2):
                    g2s.append(block_gather(pd, "g2", b, b * b_ch,
                                            ag2_out[:, :], Y2W, Y2W, 0))
                for b in range(NB2):
                    k0 = b * b_ch
                    g2 = g2s[b]
                    pe = bank1(f"pe_{b}")
                    for j in range(b_ch):
                        k = k0 + j
                        nc.tensor.matmul(
                            out=pe[0:DEMB, 0:BW],
                            lhsT=g2[:, j, 0:DEMB],
                            rhs=s_all[:, k * BW:(k + 1) * BW],
                            start=(j == 0), stop=(j == b_ch - 1),
                        )
                    bsl = slice(b * BW, (b + 1) * BW)
                    nc.scalar.copy(out=c_loc[0:DEMB, bsl], in_=pe[0:DEMB, 0:BW])
                # per-128-block tails feeding ag3_in: embT^2, -sq, bf16
                # hi/lo split (lhsT/bias work is deferred past the AG3
                # trigger so it overlaps the collective)
                for b in range(NBLK):
                    bsl = slice(b * P, (b + 1) * P)
                    sqt = pd.tile([DEMB, P], F32, tag="sqtmp", bufs=2,
                                  name=f"sqt_{b}")
                    nc.scalar.square(out=sqt[:, 0:P],
                                     in_=c_loc[0:DEMB, bsl])
                    psq = bank1(f"psq_{b}")
                    nc.tensor.matmul(
                        out=psq[0:1, 0:P],
                        lhsT=negones[:],
                        rhs=sqt[:, 0:P],
                        start=True, stop=True,
                    )
                    nc.scalar.copy(out=sqrow[0:1, bsl], in_=psq[0:1, 0:P])
                    nc.scalar.copy(out=sqh_b[0:1, bsl], in_=psq[0:1, 0:P])
                    nc.vector.tensor_tensor(
                        out=sql_b[0:1, bsl],
                        in0=psq[0:1, 0:P],
                        in1=sqh_b[0:1, bsl],
                        op=mybir.AluOpType.subtract,
                    )

                nc.sync.dma_start(out=ag3_in[0:DEMB, :], in_=c_loc[:])
                nc.sync.dma_start(out=ag3_in[DEMB:DEMB + 1, :], in_=sqh_b[:])
                nc.sync.dma_start(out=ag3_in[DEMB + 1:DEMB + 2, :], in_=sql_b[:])
                nc.gpsimd.collective_compute(
                    "AllGather", mybir.AluOpType.bypass, replica_groups=rg,
                    ins=[ag3_in.ap().opt()], outs=[ag3_out.ap().opt()],
                )
                # overlapped with AG3: lhsT rows and the bias transpose
                for b in range(NBLK):
                    bsl = slice(b * P, (b + 1) * P)
                    nc.vector.tensor_scalar_mul(lhsT_all[0:DEMB, bsl],
                                                c_loc[0:DEMB, bsl], 2.0)
                    pbt = bank1(f"pbt_{b}")
                    nc.tensor.matmul(
                        out=pbt[:, 0:1], lhsT=sqrow[0:1, bsl],
                        rhs=one1[:], start=True, stop=True,
                    )
                    nc.vector.tensor_copy(bias_t[:, b:b + 1], pbt[:, 0:1])
                nc.vector.memset(lhsT_all[DEMB:DEMB + 2, :], 1.0)
                # wt3 is a full-128-partition tile; only rows 0:66 are real
                # (dummy matmul values are discarded, the stationary just has
                # to load all 128 rows to register as PE activity)
                wt3 = sb.tile([P, 512], BF16, tag="wt3", bufs=1)
                nc.vector.memset(wt3[DEMB:P, :], 0)
                nc.sync.dma_start(out=wt3[0:DEMB + 2, :],
                                  in_=ag3_out[0:DEMB + 2, 0:512])
                warm(W_AG3, "ag3", wt3, wt3, P, 512)

        ps.release()
        if kphases >= 5:
            # ---- phase E: stripes of softmax(-dist) ------------------------
            NQ = 4          # chunks per stripe
            QW = N // NQ    # 2048 cols per chunk (2 PSUM banks x 2 bufs)
            with (
                tc.tile_pool(name="pef", bufs=1) as pef,
                tc.tile_pool(name="pse", bufs=1, space="PSUM") as pse,
            ):
                rhs_full = pef.tile([P, N], BF16, tag="rhs", bufs=1)
                nc.vector.memset(rhs_full[DEMB:P, :], 0)
                for r in range(NC):
                    nc.sync.dma_start(
                        out=rhs_full[0:DEMB + 2, r * R:(r + 1) * R],
                        in_=ag3_out[r * (DEMB + 2):(r + 1) * (DEMB + 2), :],
                    )
                for s in range(NBLK):
                    stripe = pef.tile([P, N], BF16, tag="stripe", bufs=6,
                                      name=f"stripe_{s}")
                    sums = pef.tile([P, NQ], F32, tag="sums", bufs=4,
                                    name=f"sums_{s}")
                    for q in range(NQ):
                        pz = pse.tile([P, QW], F32, tag="pz", bufs=2,
                                      name=f"pz_{s}_{q}")
                        for jj in range(QW // 512):
                            c0 = q * QW + jj * 512
                            nc.tensor.matmul(
                                out=pz[:, jj * 512:(jj + 1) * 512],
                                lhsT=lhsT_all[:, s * P:(s + 1) * P],
                                rhs=rhs_full[:, c0:c0 + 512],
                                start=True, stop=True,
                            )
                        # row sums: split between ScalarE (fused accum_out)
                        # and VectorE (reduce) so neither engine owns all 4
                        if q < 3:
                            nc.scalar.activation(
                                out=stripe[:, q * QW:(q + 1) * QW],
                                in_=pz[:],
                                func=mybir.ActivationFunctionType.Exp,
                                bias=bias_t[:, s:s + 1],
                                scale=1.0,
                                accum_out=sums[:, q:q + 1],
                            )
                        else:
                            nc.scalar.activation(
                                out=stripe[:, q * QW:(q + 1) * QW],
                                in_=pz[:],
                                func=mybir.ActivationFunctionType.Exp,
                                bias=bias_t[:, s:s + 1],
                                scale=1.0,
                            )
                            nc.vector.tensor_reduce(
                                out=sums[:, q:q + 1],
                                in_=stripe[:, q * QW:(q + 1) * QW],
                                axis=mybir.AxisListType.X,
                                op=mybir.AluOpType.add,
                            )
                    stot = pef.tile([P, 1], F32, tag="stot", bufs=2,
                                    name=f"stot_{s}")
                    nc.vector.tensor_reduce(
                        out=stot[:], in_=sums[:], axis=mybir.AxisListType.X,
                        op=mybir.AluOpType.add,
                    )
                    rec = pef.tile([P, 1], F32, tag="rec", bufs=2,
                                   name=f"rec_{s}")
                    nc.vector.reciprocal(rec[:], stot[:])
                    # normalize + store per quarter so the output DMA starts
                    # as soon as the first quarter is scaled
                    for q in range(NQ):
                        qs = slice(q * QW, (q + 1) * QW)
                        nc.vector.tensor_scalar(
                            out=stripe[:, qs],
                            in0=stripe[:, qs],
                            scalar1=rec[:, 0:1],
                            scalar2=1e-10,
                            op0=mybir.AluOpType.mult,
                            op1=mybir.AluOpType.add,
                        )
                        nc.sync.dma_start(out=out[s * P:(s + 1) * P, qs],
                                          in_=stripe[:, qs])

    nc.finalize()
    return nc


def _block_unique(c, edge_src, edge_dst, edge_val):
    """Per 256-row block: unique src rows + (slot, dstrow, val) triples."""
    out = []
    sel = (edge_dst >= c * R) & (edge_dst < (c + 1) * R)
    src = edge_src[sel]
    val = edge_val[sel]
    loc = edge_dst[sel] - c * R
    blk = loc // BW
    for b in range(NB2):
        m = blk == b
        sb_, vb, rb = src[m], val[m], loc[m] % BW
        uniq, slot = np.unique(sb_, return_inverse=True)
        out.append((uniq, slot, rb, vb))
    return out


def _prep_core(blocks, b_ch):
    """Pack per-block unique srcs into b_ch chunks; build multi-hot S."""
    c1 = NB2 * b_ch
    src_pad = np.zeros(c1 * P, np.int16)
    smat = np.zeros((c1 * P, BW), np.float32)  # [slot, dstrow]
    for b, (uniq, slot, rb, vb) in enumerate(blocks):
        assert len(uniq) <= b_ch * P
        lo = b * b_ch * P
        src_pad[lo:lo + len(uniq)] = uniq.astype(np.int16)
        np.add.at(smat, (lo + slot, rb), vb)

    idx = np.tile(np.ascontiguousarray(src_pad.reshape(-1, 16).T), (8, 1))
    # device tile layout: [slot%128 partition, chunk*BW + dstrow free]
    smat_d = np.ascontiguousarray(
        smat.reshape(c1, P, BW).transpose(1, 0, 2).reshape(P, c1 * BW)
    ).astype(F8NP)
    return idx, smat_d


def kernel(X, W1, W2, edge_val, edge_src, edge_dst):
    global LAST_RESULTS
    X = np.asarray(X, np.float32)
    W1 = np.asarray(W1, np.float32)
    W2 = np.asarray(W2, np.float32)
    edge_val = np.asarray(edge_val, np.float32)
    edge_src = np.asarray(edge_src, np.int32)
    edge_dst = np.asarray(edge_dst, np.int32)

    # chunks per (core, block) from deduped src counts, uniform across cores
    per_core_blocks = [_block_unique(c, edge_src, edge_dst, edge_val)
                       for c in range(NC)]
    b_ch = max(int(np.ceil(len(u) / P)) for blocks in per_core_blocks
               for (u, _, _, _) in blocks)
    b_ch = max(b_ch, 1)

    if b_ch not in _GRAPH_CACHE:
        _GRAPH_CACHE[b_ch] = _build(b_ch)
    nc = _GRAPH_CACHE[b_ch]

    w1b = W1.astype(BFNP)
    w2b = W2.astype(BFNP)
    in_maps = []
    for c in range(NC):
        idx, smat_d = _prep_core(per_core_blocks[c], b_ch)
        in_maps.append({
            "xT": np.ascontiguousarray(X[c * R:(c + 1) * R].T).astype(BFNP),
            "w1": w1b,
            "w2": w2b,
            "idx": idx,
            "smat": smat_d,
        })

    trace = os.environ.get("KERNEL_TRACE", "0") == "1"
    res = run_bass_kernel_spmd(nc, in_maps, core_ids=list(range(NC)), trace=trace)
    LAST_RESULTS = res
    return np.concatenate(
        [res.results[c]["out"] for c in range(NC)], axis=0
    ).astype(np.float32)
